# revision 25
# baseline (speedup 1.0000x reference)
"""Trainium2 Bass kernel for nn_AttentionModule (B=4, N=4096, M=4096, D=1024).

reference:
    s = einsum('bnd,bmd->bnm', q, a)      # [B,N,M]
    e = softmax(s, axis=1)                # over n
    h = einsum('bnm,bnd->bmd', e, q)      # [B,M,D]

This is standard attention with query=a, key=value=q (softmax over keys n):
    h[b,m,:] = sum_n softmax_n(a[b,m]·q[b,n]) q[b,n,:]

Sharding: 8 cores = batch(4) x M-halves(2). Zero collectives.
Per core: S' = A_loc @ Q^T [2048,4096], row-softmax, h_loc = P @ Q [2048,1024].

Per-core loop: n-blocks (8 x 512) outer, m-tiles (16 x 128) inner.
mm1 contracts over d (operands pre-transposed on host: qT, aT d-major).
Exact online softmax over the free axis n: running row max; on a max
increase the running h and Z are rescaled by exp(m_old - m_new) (rescale
multiply on ACT, adds on DVE) -- overflow-proof for any data. exp on ACT
with fused row-sum (accum_out). P is PE-transposed (via identity matmul)
to n-major for mm2 (contraction over n), which uses q in natural layout.
h accumulates in SBUF via DVE adds; final h /= Z then DMA out, inlined
per m-tile. All PE operands are fp16 (11-bit mantissa, same as tf32;
inputs cast on host): 1 cycle/row at moving dim 512, separate LDWEIGHTS
instructions the PE reorder window hides, FWL-eligible, half the DMA
of fp32. PSUM accumulation is fp32 throughout; one-iteration software
pipelining keeps the PE stream dense (mm1 of iter i+1 issues before the
softmax/transpose/mm2 of iter i).
"""

import sys

for _p in ("/opt/trn_rl_repo/concourse", "/opt/trn_rl_repo"):
    if _p not in sys.path:
        sys.path.insert(0, _p)

import numpy as np

import concourse.bass as bass
import concourse.tile as tile
from concourse import bacc, mybir, masks
from concourse import bass_utils

B, N, M, D = 4, 4096, 4096, 1024
NCORES = 8
MLOC = M // 2          # m per core
NB = 512               # n block width
NBC = N // NB          # 8 n blocks
MT = MLOC // 128       # 16 m tiles
DC = D // 128          # 8 d chunks

F32 = mybir.dt.float32
F32R = mybir.dt.float32r
F16 = mybir.dt.float16


def build_nc(repeat=None):
    """repeat=None: plain kernel. repeat=R: whole body wrapped in a
    hardware For_i loop executing R times (R may be 0) -- used only for
    wall-clock timing amplification."""
    nc = bacc.Bacc("TRN2", target_bir_lowering=False, debug=False,
                   num_devices=NCORES)
    qt = nc.dram_tensor("qt", [D, N], F16, kind="ExternalInput").ap()
    at = nc.dram_tensor("at", [D, MLOC], F16, kind="ExternalInput").ap()
    qn = nc.dram_tensor("qn", [N, D], F16, kind="ExternalInput").ap()
    h = nc.dram_tensor("h", [MLOC, D], F32, kind="ExternalOutput").ap()
    qt_r = qt
    at_r = at
    qn_r = qn

    with tile.TileContext(nc) as tc:
        from contextlib import ExitStack
        ctx = ExitStack()
        with ctx:
            p_at = ctx.enter_context(tc.tile_pool(name="p_at", bufs=1))
            p_h = ctx.enter_context(tc.tile_pool(name="p_h", bufs=1))
            p_qt = ctx.enter_context(tc.tile_pool(name="p_qt", bufs=2))
            p_qn = ctx.enter_context(tc.tile_pool(name="p_qn", bufs=2))
            p_p = ctx.enter_context(tc.tile_pool(name="p_p", bufs=3))
            p_pt = ctx.enter_context(tc.tile_pool(name="p_pt", bufs=3))
            p_stat = ctx.enter_context(tc.tile_pool(name="p_stat", bufs=1))
            p_tmp = ctx.enter_context(tc.tile_pool(name="p_tmp", bufs=4))
            ps_s = ctx.enter_context(
                tc.tile_pool(name="ps_s", bufs=2, space="PSUM"))
            ps_t = ctx.enter_context(
                tc.tile_pool(name="ps_t", bufs=2, space="PSUM"))
            ps_h = ctx.enter_context(
                tc.tile_pool(name="ps_h", bufs=2, space="PSUM"))

            # persistent tiles
            at_sb = p_at.tile([128, DC, MLOC], F16)        # 64KB/p
            h_sb = p_h.tile([128, MT, D], F32)              # 64KB/p
            ident = p_stat.tile([128, 128], F16)
            mrun = p_stat.tile([128, MT], F32)              # running row max
            zrun = p_stat.tile([128, MT], F32)              # running row sum
            masks.make_identity(nc, ident[:])

            loop_cm = (tc.For_i(0, repeat, 1) if repeat is not None
                       else None)
            if loop_cm is not None:
                loop_cm.__enter__()

            # software pipeline state
            qt_sb = None
            qn_sb = None
            pending = None  # (P_sb tile, nb, mt) awaiting phase2

            def load_nb(nb):
                nonlocal qt_sb, qn_sb
                qt_sb = p_qt.tile([128, DC, NB], F16)
                for c in range(DC):
                    nc.sync.dma_start(
                        qt_sb[:, c, :],
                        qt_r[128 * c:128 * (c + 1), NB * nb:NB * (nb + 1)])
                qn_sb = p_qn.tile([128, 4, D], F16)
                for k in range(4):
                    r0 = NB * nb + 128 * k
                    nc.sync.dma_start(qn_sb[:, k, :], qn_r[r0:r0 + 128, :])

            # DMA priority: interleave first n-block's q tiles with A^T
            # chunks so the first accumulation chain starts after ~1.3MB.
            qt_sb = p_qt.tile([128, DC, NB], F16)
            qn_sb = p_qn.tile([128, 4, D], F16)
            for c in range(DC):
                nc.sync.dma_start(
                    qt_sb[:, c, :], qt_r[128 * c:128 * (c + 1), 0:NB])
                nc.sync.dma_start(at_sb[:, c, 0:128],
                                  at_r[128 * c:128 * (c + 1), 0:128])
            for k in range(4):
                nc.sync.dma_start(qn_sb[:, k, :], qn_r[128 * k:128 * (k + 1), :])
            for m0, m1 in ((128, 512), (512, 1024), (1024, 1536),
                           (1536, MLOC)):
                for c in range(DC):
                    nc.sync.dma_start(at_sb[:, c, m0:m1],
                                      at_r[128 * c:128 * (c + 1), m0:m1])

            def phase1(nb, mt):
                nonlocal qt_sb, qn_sb
                if mt == 0 and nb > 0:
                    load_nb(nb)

                s_ps = ps_s.tile([128, NB], F32)
                for c in range(DC):
                    nc.tensor.matmul(
                        s_ps[:],
                        at_sb[:, c, 128 * mt:128 * (mt + 1)],
                        qt_sb[:, c, :],
                        start=(c == 0), stop=(c == DC - 1))

                # online softmax: running max with h/Z rescale (exact)
                r = p_tmp.tile([128, 1], F32)
                nc.vector.reduce_max(r[:], s_ps[:], axis=mybir.AxisListType.X)
                nmnew = p_tmp.tile([128, 1], F32)
                scale = None
                if nb == 0:
                    nc.vector.tensor_copy(mrun[:, mt:mt + 1], r[:])
                    nc.scalar.activation(
                        nmnew[:], r[:],
                        mybir.ActivationFunctionType.Copy, scale=-1.0)
                else:
                    mnew = p_tmp.tile([128, 1], F32)
                    nc.vector.tensor_tensor(
                        out=mnew[:], in0=mrun[:, mt:mt + 1], in1=r[:],
                        op=mybir.AluOpType.max)
                    nc.scalar.activation(
                        nmnew[:], mnew[:],
                        mybir.ActivationFunctionType.Copy, scale=-1.0)
                    scale = p_tmp.tile([128, 1], F32)
                    # scale = exp(m_old - m_new)
                    nc.scalar.activation(
                        scale[:], mrun[:, mt:mt + 1],
                        mybir.ActivationFunctionType.Exp, bias=nmnew[:])
                    nc.vector.tensor_copy(mrun[:, mt:mt + 1], mnew[:])

                p_sb = p_p.tile([128, NB], F16)
                zp = p_tmp.tile([128, 1], F32)
                nc.scalar.activation(
                    p_sb[:], s_ps[:],
                    mybir.ActivationFunctionType.Exp,
                    bias=nmnew[:], scale=1.0,
                    accum_out=zp[:])
                if nb == 0:
                    nc.vector.tensor_copy(zrun[:, mt:mt + 1], zp[:])
                else:
                    # Z = Z*scale + zp
                    nc.vector.tensor_scalar(
                        out=zrun[:, mt:mt + 1], in0=zrun[:, mt:mt + 1],
                        scalar1=scale[:], scalar2=zp[:],
                        op0=mybir.AluOpType.mult, op1=mybir.AluOpType.add)
                return (p_sb, nb, mt, qn_sb, scale)

            def phase2(state):
                p_sb, nb, mt, qn_tile, scale = state
                pt_ps = ps_t.tile([128, NB], F16)
                for k in range(4):
                    nc.tensor.transpose(
                        pt_ps[:, 128 * k:128 * (k + 1)],
                        p_sb[:, 128 * k:128 * (k + 1)],
                        ident[:])
                pt_sb = p_pt.tile([128, NB], F16)
                nc.vector.tensor_copy(pt_sb[:], pt_ps[:])

                h_ps = ps_h.tile([128, D], F32)
                for k in range(4):
                    for db in range(2):
                        nc.tensor.matmul(
                            h_ps[:, 512 * db:512 * (db + 1)],
                            pt_sb[:, 128 * k:128 * (k + 1)],
                            qn_tile[:, k, 512 * db:512 * (db + 1)],
                            start=(k == 0), stop=(k == 3))
                if nb == 0:
                    nc.vector.tensor_copy(h_sb[:, mt, :], h_ps[:])
                else:
                    # h = h*scale + h_ps  (rescale on ACT, add on DVE)
                    nc.scalar.activation(
                        h_sb[:, mt, :], h_sb[:, mt, :],
                        mybir.ActivationFunctionType.Copy, scale=scale[:])
                    nc.vector.tensor_add(h_sb[:, mt, :], h_sb[:, mt, :],
                                         h_ps[:])
                if nb == NBC - 1:
                    # inline epilogue: h /= Z, DMA out
                    rz = p_tmp.tile([128, 1], F32)
                    nc.vector.reciprocal(rz[:], zrun[:, mt:mt + 1])
                    nc.vector.tensor_scalar_mul(h_sb[:, mt, :],
                                                h_sb[:, mt, :], rz[:])
                    nc.sync.dma_start(h[128 * mt:128 * (mt + 1), :],
                                      h_sb[:, mt, :])

            for nb in range(NBC):
                for mt in range(MT):
                    st = phase1(nb, mt)
                    if pending is not None:
                        phase2(pending)
                    pending = st
            phase2(pending)
            pending = None

            if loop_cm is not None:
                loop_cm.__exit__(None, None, None)

    nc.compile()
    return nc


_NC_CACHE = None


def _get_nc():
    global _NC_CACHE
    if _NC_CACHE is None:
        _NC_CACHE = build_nc()
    return _NC_CACHE


def make_in_maps(q, a):
    q = np.ascontiguousarray(q, dtype=np.float32)
    a = np.ascontiguousarray(a, dtype=np.float32)
    in_maps = []
    for c in range(NCORES):
        b, j = divmod(c, 2)
        in_maps.append({
            "qt": np.ascontiguousarray(q[b].T).astype(np.float16),
            "at": np.ascontiguousarray(
                a[b, j * MLOC:(j + 1) * MLOC].T).astype(np.float16),
            "qn": q[b].astype(np.float16),
        })
    return in_maps


def assemble(results):
    h = np.empty((B, M, D), dtype=np.float32)
    for c in range(NCORES):
        b, j = divmod(c, 2)
        h[b, j * MLOC:(j + 1) * MLOC] = results[c]["h"]
    return h


def kernel(q, a):
    import os
    # the axon NTFF profile hook is unavailable in this container;
    # force trace off so a stray BASS_TRACE env can't crash the run
    os.environ["BASS_NEVER_TRACE"] = "1"
    nc = _get_nc()
    in_maps = make_in_maps(q, a)
    res = bass_utils.run_bass_kernel_spmd(nc, in_maps,
                                          core_ids=list(range(NCORES)))
    return assemble(res.results)
